# revision 4
# baseline (speedup 1.0000x reference)
"""DirectionalGINConv (eps=0) Trainium2 kernel, 8-core SPMD.

  agg_i = sum_{j->i} x_j ; out = relu((x + agg) @ W.T + b)   (relu o relu = relu)

Strategy (hardcoded for N=50000, E=800000, C=64, 8 cores):
  - Destination-node sharding: core c owns dst rows [c*6250, (c+1)*6250).
  - Host re-packs each core's 6250 nodes into 49 blocks of <=128 slots,
    balancing per-block edge counts so every block needs exactly 16 gather
    tiles (8 per src-half; M data-dependent "overflow" blocks get 17) --
    0.35% gather padding vs the edge count.
  - src halves: the dma_gather idx is int16 (32768 rows max) -> half A
    table = x rows [0, 32768), half B = rows [17232, 50000); flex edges
    (src in the overlap) are assigned so half A gets exactly 1024 edges
    per block (so only the last B tile of a block has pads).
  - Device per core: dma_gather 128-byte elements (64ch fp16 rows read
    from a 256B-stride table; bass's %256 elem assert is a transpose-path
    restriction, bypassed by emitting InstDMAGatherAnt directly). Calls
    are kept small (~20 tiles) and round-robined over all 4 SWDGE queues:
    the gather is bound at ~2.4ns/element by SDMA descriptor processing
    (measured; insensitive to element size, source memory, HBM locality,
    and packet mode), so the whole kernel is organized to keep the 4
    queues saturated from ~8us into the run until the end.
  - Per block: one-hot S[e, slot] built on DVE (is_equal vs a 3-D iota
    constant); segment-sum via PE psum accumulation (16-17 matmuls of
    [128e,64ch]^T @ [128e,128slot]); h = psum + x_shard.T (DVE); MLP
    matmul (W.T stationary); relu+bias on ACT written channel-major into
    a wide out buffer; per-chunk slab DMAs out. Host transposes and
    un-permutes the output (block packing is a host-known permutation).
"""

import numpy as np
from contextlib import ExitStack

N_NODES = 50000
IN_CH = 64
OUT_CH = 64
N_CORES = 8
SHARD = N_NODES // N_CORES          # 6250
P = 128
NBLK = (SHARD + P - 1) // P         # 49 blocks
SBUF_TABLE = True                   # gather from SBUF-resident x (vs HBM)
BASE1 = 17280 if SBUF_TABLE else 17232   # half B table base (stripe-aligned for SBUF)
STRIPES = (N_NODES + P - 1) // P    # 391 SBUF table stripes
KA = 8                              # tiles per block, half A (1024 edges)
KB0 = 8                             # tiles per block, half B (non-overflow)
CA = KA * P                         # 1024 half-A edges per block


def _pack_blocks(deg, caps):
    """Balance nodes into NBLK blocks of <=128 slots by edge count.

    deg: [SHARD] per-node edge counts. caps: [NBLK] per-block edge targets.
    Round-matching: nodes sorted by degree desc, one per block per round,
    heaviest node -> most remaining capacity; then swap-fix any block left
    over its cap. Returns (blk_of, slot_of) or None if caps infeasible.
    """
    order = np.argsort(-deg, kind="stable")
    rem = caps.astype(np.int64).copy()
    blk_of = np.empty(SHARD, np.int64)
    slot_of = np.empty(SHARD, np.int64)
    pos = 0
    rnd = 0
    while pos < SHARD:
        take = min(NBLK, SHARD - pos)
        nodes = order[pos:pos + take]
        border = np.argsort(-rem, kind="stable")[:take]
        blk_of[nodes] = border
        slot_of[nodes] = rnd
        rem[border] -= deg[nodes]
        pos += take
        rnd += 1

    cnt = np.bincount(blk_of, weights=deg, minlength=NBLK).astype(np.int64)
    for _ in range(1000):
        over = int(np.argmax(cnt - caps))
        if cnt[over] <= caps[over]:
            return blk_of, slot_of
        under = int(np.argmin(cnt - caps))
        need = cnt[over] - caps[over]
        slack = caps[under] - cnt[under]
        ni = np.flatnonzero(blk_of == over)
        nj = np.flatnonzero(blk_of == under)
        delta = deg[ni][:, None] - deg[nj][None, :]   # edges moved by swap
        delta = np.where((delta >= 1) & (delta <= slack), delta, -1)
        if delta.max() < 1:
            return None
        # prefer the smallest sufficient move, else the largest available
        want = np.where(delta >= min(need, slack), delta, 10 ** 9)
        i, j = np.unravel_index(
            np.argmin(want) if want.min() < 10 ** 9 else np.argmax(delta),
            delta.shape)
        a, b_ = ni[i], nj[j]
        blk_of[a], blk_of[b_] = under, over
        slot_of[a], slot_of[b_] = slot_of[b_], slot_of[a]
        d = deg[a] - deg[b_]
        cnt[over] -= d
        cnt[under] += d
    return None


def _route(src, dst):
    """Host routing. Returns per-core idx/slot arrays, col_node maps, M."""
    src = np.asarray(src, np.int64)
    dst = np.asarray(dst, np.int64)
    core = dst // SHARD
    l = dst - core * SHARD
    deg_all = np.bincount(dst, minlength=N_NODES).reshape(N_CORES, SHARD)
    edges_per_core = deg_all.sum(axis=1)

    M = max(0, int(np.max(-(-(edges_per_core - NBLK * (CA + KB0 * P)) // P))))
    for _try in range(6):
        caps = np.array([CA + (KB0 + 1) * P] * M + [CA + KB0 * P] * (NBLK - M),
                        np.int64)
        packs = [_pack_blocks(deg_all[c], caps) for c in range(N_CORES)]
        ok = True
        for c in range(N_CORES):
            blk_of, _ = packs[c]
            cnt_b = np.bincount(blk_of, weights=deg_all[c], minlength=NBLK)
            if (cnt_b > caps).any():
                ok = False
                break
        if ok:
            break
        M += 1
    assert ok, "block packing failed"

    KBs = np.array([KB0 + 1 if b < M else KB0 for b in range(NBLK)], np.int64)
    TA = NBLK * KA
    TB = int(KBs.sum())
    LA, LB = TA * P, TB * P
    tbaseB = np.concatenate([[0], np.cumsum(KBs)])[:-1]  # B tile offset per blk

    # per-edge block/slot
    blk = np.empty(len(src), np.int64)
    slot = np.empty(len(src), np.int64)
    col_nodes = []
    for c in range(N_CORES):
        m = core == c
        blk_of, slot_of = packs[c]
        blk[m] = blk_of[l[m]]
        slot[m] = slot_of[l[m]]
        cn = np.full(NBLK * P, -1, np.int64)
        cn[blk_of * P + slot_of] = np.arange(SHARD)
        col_nodes.append(cn)

    gid = core * NBLK + blk
    ngrp = N_CORES * NBLK
    cat = np.where(src < BASE1, 0, np.where(src < 32768, 1, 2)).astype(np.int64)

    n_lo = np.bincount(gid[cat == 0], minlength=ngrp)
    n_flex = np.bincount(gid[cat == 1], minlength=ngrp)
    f0 = CA - n_lo          # flex edges sent to half A
    assert (f0 >= 0).all() and (f0 <= n_flex).all(), "half-A quota infeasible"

    # rank within (gid, cat) ordered by src, to split flex
    key_gc = gid * 3 + cat
    order1 = np.lexsort((src, key_gc))
    sk = key_gc[order1]
    starts = np.r_[0, np.flatnonzero(sk[1:] != sk[:-1]) + 1]
    start_of = np.zeros(ngrp * 3, np.int64)
    start_of[sk[starts]] = starts
    rank_gc = np.empty_like(order1)
    rank_gc[order1] = np.arange(len(order1)) - start_of[key_gc][order1]
    half = np.where(cat == 0, 0,
                    np.where(cat == 2, 1, (rank_gc >= f0[gid]).astype(np.int64)))

    # rank within (gid, half) ordered by src
    key_gh = gid * 2 + half
    order2 = np.lexsort((src, key_gh))
    sk2 = key_gh[order2]
    starts2 = np.r_[0, np.flatnonzero(sk2[1:] != sk2[:-1]) + 1]
    start_of2 = np.zeros(ngrp * 2, np.int64)
    start_of2[sk2[starts2]] = starts2
    rank = np.empty_like(order2)
    rank[order2] = np.arange(len(order2)) - start_of2[key_gh][order2]

    # positions
    spreadA = ((np.arange(LA, dtype=np.int64) * 9973) % 32768).astype(np.int16)
    spreadB = ((np.arange(LB, dtype=np.int64) * 9973) % (N_NODES - BASE1)).astype(np.int16)
    idxA = np.tile(spreadA, (N_CORES, 1))
    idxB = np.tile(spreadB, (N_CORES, 1))
    slotA = np.full((N_CORES, LA), -1.0, np.float32)
    slotB = np.full((N_CORES, LB), -1.0, np.float32)

    mA = half == 0
    posA = blk[mA] * (KA * P) + rank[mA]
    assert (rank[mA] < CA).all()
    idxA[core[mA], posA] = src[mA].astype(np.int16)
    slotA[core[mA], posA] = slot[mA].astype(np.float32)

    mB = half == 1
    posB = tbaseB[blk[mB]] * P + rank[mB]
    assert (rank[mB] < KBs[blk[mB]] * P).all()
    idxB[core[mB], posB] = (src[mB] - BASE1).astype(np.int16)
    slotB[core[mB], posB] = slot[mB].astype(np.float32)

    return idxA, idxB, slotA, slotB, col_nodes, M


def _wrap_idx(idx):
    """[L] int16 -> [128, L/16] wrapped (i -> [i%16, i//16]) + replicated."""
    w = idx.reshape(-1, 16).T
    return np.ascontiguousarray(np.tile(w, (8, 1)))


def _slot_tiles(slots, f16):
    """[L] -> [128, L/128] (col t = edges t*128..t*128+127), cast to f16."""
    return np.ascontiguousarray(slots.reshape(-1, P).T).astype(f16)


def _raw_gather(eng, out_ap, in_ap, idxs_ap, num_idxs, elem_size,
                queue_num, src_sbuf=False, elem_step=128, single_packet=False,
                num_idxs_reg=None):
    """nc.gpsimd.dma_gather minus the %256 elem-size assert (non-transpose).

    The 256B granularity is a transpose-path restriction; the firmware's
    non-transpose path reads elem_size_bytes per index either from an HBM
    table (row stride elem_step, a 256B multiple) or from an SBUF-resident
    token table (token i -> partition i%128, stripe i//128).
    """
    import concourse.mybir as mybir
    from concourse import ap_utils
    from concourse._compat import round_up_to_multiple, exact_div
    eng._assert_queue_num(queue_num)
    assert idxs_ap.dtype == mybir.dt.int16
    assert in_ap.dtype == out_ap.dtype
    assert ap_utils.ap_is_contiguous(out_ap.ap[1:])
    assert ap_utils.ap_is_contiguous(idxs_ap.ap[1:])
    assert out_ap.ap[-1][1] == elem_size
    assert out_ap.ap[0][1] * out_ap.ap[1][1] == round_up_to_multiple(num_idxs, 128)
    if src_sbuf:
        _in_ap = [eng.lower_ap(in_ap)]
        stride_bytes_256 = 0
        sb = dict(sbuf_tokens_per_rank=128,
                  sbuf_free_dim_per_rank=elem_size * mybir.dt.size(in_ap.dtype),
                  sbuf_free_dim_pad_per_rank=0, sbuf_byte_offset=0)
    else:
        assert in_ap.ap[-1][1] == elem_size
        assert in_ap.ap[0][0] == elem_step
        stride_bytes = elem_step * mybir.dt.size(in_ap.dtype)
        stride_bytes_256 = exact_div(stride_bytes, 256)
        assert 0 < stride_bytes_256 < 256
        _in_ap = eng.lower_ap_dma(in_ap, for_custom_bir_dma=True)
        sb = dict(sbuf_tokens_per_rank=0, sbuf_free_dim_per_rank=0,
                  sbuf_free_dim_pad_per_rank=0, sbuf_byte_offset=0)
    _idxs_ap = eng.lower_ap(idxs_ap)
    _out_ap = eng.lower_ap(out_ap)
    return eng.add_instruction(
        mybir.InstDMAGatherAnt(
            name=eng.bass.get_next_instruction_name(),
            ins=[*_in_ap, _idxs_ap, eng.lower_val_access(
                eng.to_reg(num_idxs if num_idxs_reg is None else num_idxs_reg))],
            outs=[_out_ap],
            transpose=False,
            num_idxs=num_idxs,
            elem_size=elem_size,
            stride_bytes_256=stride_bytes_256,
            gen_mode=0,
            single_packet=single_packet,
            queue_num=queue_num,
            **sb,
        )
    )


def _build_program(M):
    import concourse.bacc as bacc
    import concourse.tile as tile
    import concourse.mybir as mybir
    from concourse import library_config

    f16 = mybir.dt.float16
    f32 = mybir.dt.float32
    i16 = mybir.dt.int16

    KBs = [KB0 + 1 if b < M else KB0 for b in range(NBLK)]
    TA = NBLK * KA
    TB = sum(KBs)
    LA, LB = TA * P, TB * P
    tbaseB = [0]
    for b in range(NBLK):
        tbaseB.append(tbaseB[-1] + KBs[b])

    CHUNKS = [5] * 9 + [4]
    assert sum(CHUNKS) == NBLK
    CBMAX = max(CHUNKS)
    TCMAX = CBMAX * (KB0 + 1)           # max tiles per chunk-half

    nc = bacc.Bacc("TRN2", target_bir_lowering=False, debug=False,
                   num_devices=N_CORES, num_swdge_queues=4)
    if SBUF_TABLE:
        xs_d = nc.dram_tensor("xs", [128, STRIPES * IN_CH], f16,
                              kind="ExternalInput")
    else:
        xg_d = nc.dram_tensor("xg", [N_NODES, 128], f16, kind="ExternalInput")
    i0_d = nc.dram_tensor("i0", [128, LA // 16], i16, kind="ExternalInput")
    i1_d = nc.dram_tensor("i1", [128, LB // 16], i16, kind="ExternalInput")
    s0_d = nc.dram_tensor("s0", [P, TA], f16, kind="ExternalInput")
    s1_d = nc.dram_tensor("s1", [P, TB], f16, kind="ExternalInput")
    xt_d = nc.dram_tensor("xt", [IN_CH, NBLK * P], f32, kind="ExternalInput")
    wt_d = nc.dram_tensor("wt", [IN_CH, OUT_CH], f32, kind="ExternalInput")
    b_d = nc.dram_tensor("b", [OUT_CH, 1], f32, kind="ExternalInput")
    iota_d = nc.dram_tensor("iota", [P, KB0 + 1, P], f16, kind="ExternalInput")
    och_d = nc.dram_tensor("och", [OUT_CH, NBLK * P], f32, kind="ExternalOutput")

    with tile.TileContext(nc) as tc, ExitStack() as ctx:
        const_p = ctx.enter_context(tc.tile_pool(name="const", bufs=1))
        gat_p = ctx.enter_context(tc.tile_pool(name="gat", bufs=6))
        sel_p = ctx.enter_context(tc.tile_pool(name="sel", bufs=4))
        h_p = ctx.enter_context(tc.tile_pool(name="h", bufs=3))
        psum_agg = ctx.enter_context(tc.tile_pool(name="pagg", bufs=4, space="PSUM"))
        psum_mlp = ctx.enter_context(tc.tile_pool(name="pmlp", bufs=2, space="PSUM"))

        nc.gpsimd.load_library(library_config.mlp)

        if SBUF_TABLE:
            xs_t = const_p.tile([128, STRIPES * IN_CH], f16)
            nc.scalar.dma_start(out=xs_t[:], in_=xs_d.ap()[:])
        i0_t = const_p.tile([128, LA // 16], i16)
        i1_t = const_p.tile([128, LB // 16], i16)
        s0_t = const_p.tile([P, TA], f16)
        s1_t = const_p.tile([P, TB], f16)
        xt_t = const_p.tile([IN_CH, NBLK * P], f32)
        wt_t = const_p.tile([IN_CH, OUT_CH], f32)
        b_t = const_p.tile([OUT_CH, 1], f32)
        iota_t = const_p.tile([P, KB0 + 1, P], f16)
        och_t = const_p.tile([OUT_CH, NBLK * P], f32)
        # priority: first chunk's idx slices land first so gathers start early
        tA_c0 = CHUNKS[0] * KA * 8
        tB_c0 = tbaseB[CHUNKS[0]] * 8
        nc.sync.dma_start(out=i0_t[:, :tA_c0], in_=i0_d.ap()[:, :tA_c0])
        nc.sync.dma_start(out=i1_t[:, :tB_c0], in_=i1_d.ap()[:, :tB_c0])
        nc.sync.dma_start(out=i0_t[:, tA_c0:], in_=i0_d.ap()[:, tA_c0:])
        nc.sync.dma_start(out=i1_t[:, tB_c0:], in_=i1_d.ap()[:, tB_c0:])
        for t, d in [(s0_t, s0_d), (s1_t, s1_d), (iota_t, iota_d),
                     (xt_t, xt_d), (wt_t, wt_d), (b_t, b_d)]:
            nc.scalar.dma_start(out=t[:], in_=d.ap()[:])

        if SBUF_TABLE:
            tables = [xs_t[:], xs_t[:, (BASE1 // P) * IN_CH:]]
        else:
            tables = [xg_d.ap()[:, :IN_CH], xg_d.ap()[BASE1:, :IN_CH]]
        idx_tiles = [i0_t, i1_t]
        slot_tiles = [s0_t, s1_t]

        # hoist the distinct num_idxs constants into registers once, so the
        # gather dispatch stream has no interleaved MOVEs
        nparts = set()
        b0 = 0
        for CB in CHUNKS:
            for lo, hi in ((b0 * KA, (b0 + CB) * KA),
                           (tbaseB[b0], tbaseB[b0 + CB])):
                nt = hi - lo
                nparts.add((nt // 2) * P)
                nparts.add((nt - nt // 2) * P)
            b0 += CB
        nreg = {n: nc.gpsimd.to_reg(n) for n in sorted(nparts) if n > 0}
        qn = 0
        blk0 = 0
        for c, CB in enumerate(CHUNKS):
            tA0, tA1 = blk0 * KA, (blk0 + CB) * KA
            tB0, tB1 = tbaseB[blk0], tbaseB[blk0 + CB]
            spans = [(tA0, tA1), (tB0, tB1)]
            g = []
            for h in (0, 1):
                t0, t1 = spans[h]
                nt = t1 - t0
                gt = gat_p.tile([P, TCMAX, IN_CH], f16, tag=f"g{h}",
                                name=f"g{h}")
                # split each half-chunk across 2 SWDGE queues: calls must be
                # small enough that decode-side ring await_space never blocks
                # the engine (big calls serialize dispatch of later calls).
                base_t = nt // 2
                p0 = 0
                for part in range(2):
                    tt = base_t if part == 0 else nt - base_t
                    if tt <= 0:
                        continue
                    n_part = tt * P
                    col0 = (t0 + p0) * 8
                    _raw_gather(nc.gpsimd, gt[:, p0:p0 + tt, :], tables[h],
                                idx_tiles[h][:, col0: col0 + n_part // 16],
                                n_part, IN_CH, qn % 4, src_sbuf=SBUF_TABLE,
                                num_idxs_reg=nreg[n_part])
                    qn += 1
                    p0 += tt
                g.append(gt)
            for bl in range(CB):
                blk = blk0 + bl
                KB = KBs[blk]
                S_blk = []
                for h, K, col0 in ((0, KA, blk * KA),
                                   (1, KB, tbaseB[blk])):
                    S = sel_p.tile([P, KB0 + 1, P], f16, name=f"S{h}",
                                   tag=f"S{h}")
                    nc.vector.tensor_tensor(
                        out=S[:, :K, :],
                        in0=slot_tiles[h][:, col0:col0 + K]
                            .to_broadcast([P, K, P]),
                        in1=iota_t[:, :K, :],
                        op=mybir.AluOpType.is_equal,
                    )
                    S_blk.append(S)
                pa = psum_agg.tile([IN_CH, P], f32, space="PSUM")
                n_mm = KA + KB
                mm = 0
                for hh, (h, K, tb) in enumerate((
                        (0, KA, blk * KA - tA0), (1, KB, tbaseB[blk] - tB0))):
                    for k in range(K):
                        nc.tensor.matmul(
                            out=pa[:],
                            lhsT=g[h][:, tb + k, :],
                            rhs=S_blk[hh][:, k, :],
                            start=(mm == 0),
                            stop=(mm == n_mm - 1),
                        )
                        mm += 1
                h_t = h_p.tile([IN_CH, P], f32)
                nc.vector.tensor_add(out=h_t[:], in0=pa[:],
                                     in1=xt_t[:, blk * P:(blk + 1) * P])
                pm = psum_mlp.tile([OUT_CH, P], f32, space="PSUM")
                nc.tensor.matmul(out=pm[:], lhsT=wt_t[:], rhs=h_t[:],
                                 start=True, stop=True)
                nc.scalar.activation(out=och_t[:, blk * P:(blk + 1) * P],
                                     in_=pm[:],
                                     func=mybir.ActivationFunctionType.Relu,
                                     bias=b_t[:])
            nc.sync.dma_start(out=och_d.ap()[:, blk0 * P:(blk0 + CB) * P],
                              in_=och_t[:, blk0 * P:(blk0 + CB) * P])
            blk0 += CB

    nc.compile()
    return nc


def _prepare(x, edge_index, W, b):
    """Host-side routing + per-core input maps. Returns (in_maps, col_nodes, M)."""
    f16np = np.float16
    x = np.asarray(x, np.float32)
    W = np.asarray(W, np.float32)
    b = np.asarray(b, np.float32)
    src = np.asarray(edge_index[0])
    dst = np.asarray(edge_index[1])

    idxA, idxB, slotA, slotB, col_nodes, M = _route(src, dst)
    if SBUF_TABLE:
        xs = np.zeros((128, STRIPES * IN_CH), f16np)
        xf = x.astype(f16np)
        for s in range(STRIPES):
            rows = xf[s * P:(s + 1) * P]
            xs[:len(rows), s * IN_CH:(s + 1) * IN_CH] = rows
    else:
        xg = np.zeros((N_NODES, 128), f16np)
        xg[:, :IN_CH] = x.astype(f16np)
    iota = np.broadcast_to(np.arange(P, dtype=np.float32), (P, KB0 + 1, P))
    iota = np.ascontiguousarray(iota).astype(f16np)
    wt = np.ascontiguousarray(W.T)
    b2 = np.ascontiguousarray(b.reshape(-1, 1))

    in_maps = []
    for c in range(N_CORES):
        cn = col_nodes[c]
        valid = cn >= 0
        xt = np.zeros((IN_CH, NBLK * P), np.float32)
        xt[:, valid] = x[c * SHARD + cn[valid]].T
        in_maps.append({
            ("xs" if SBUF_TABLE else "xg"): (xs if SBUF_TABLE else xg),
            "i0": _wrap_idx(idxA[c]),
            "i1": _wrap_idx(idxB[c]),
            "s0": _slot_tiles(slotA[c], f16np),
            "s1": _slot_tiles(slotB[c], f16np),
            "xt": np.ascontiguousarray(xt),
            "wt": wt,
            "b": b2,
            "iota": iota,
        })
    return in_maps, col_nodes, M


_CACHE = {}


def _get_program(M):
    if M not in _CACHE:
        _CACHE[M] = _build_program(M)
    return _CACHE[M]


def _best_effort_device_reset():
    """If a previous process wedged the NeuronCores, a reset lets this
    process's run succeed. Harmless (rc=0, state-free) on a healthy device."""
    try:
        import ctypes, jax
        jax.devices()
        lib = ctypes.CDLL("/opt/axon/libaxon_pjrt.so")
        lib.axon_reset.restype = ctypes.c_int64
        lib.axon_reset()
    except Exception:
        pass


def run(x, edge_index, W, b, trace=False):
    from concourse.bass_utils import run_bass_kernel_spmd
    _best_effort_device_reset()
    in_maps, col_nodes, M = _prepare(x, edge_index, W, b)
    nc = _get_program(M)
    res = run_bass_kernel_spmd(nc, in_maps, core_ids=list(range(N_CORES)),
                               trace=trace)
    out = np.empty((N_NODES, OUT_CH), np.float32)
    for c in range(N_CORES):
        och = res.results[c]["och"]          # [64, NBLK*P]
        cn = col_nodes[c]
        valid = cn >= 0
        out[c * SHARD + cn[valid]] = och[:, valid].T
    return out, res


def kernel(x, edge_index, W, b):
    out, _ = run(x, edge_index, W, b, trace=False)
    return out
